# revision 58
# baseline (speedup 1.0000x reference)
"""Trainium2 Bass kernel for nn_DCTExtractor.

Reference computation:
  - stego [8, 3, 1024, 1024] f32; per 8x8 block 2D DCT-II (D @ X @ D^T).
  - bits[i] = abs(round_half_even(dct[b,c,nh,nw,bh,bw])) % 2 for 1572864
    index tuples.
  - out [8, num_bits]: out[b_idx[i], i] = bits[i]; other rows 0.

Sharding: data-parallel over batch b across the 8 NeuronCores; core b
processes image b and produces output row b.

Fast path (canonical meshgrid indices): each needed coefficient is a
Frobenius inner product <X_block, D_bh (x) D_bw>.  The kernel is
HBM-bandwidth bound, so the host ships each image as 3 bytes/pixel:
an fp16 hi plane and an fp8e4m3 plane of the residual*128, both in a
layout with the two within-block axes on partitions:

    x16/r8[c, (q,j) 128, (gh, k, g', nw) 8192]   nh = (gh*4+g')*16 + q,
                                                 w = nw*8 + k

The fp16 stationary packs hi/lo weight planes side by side (the lo
plane pre-scaled by 512)

    W16[k][(q,j), 0:64]   = fp16_hi(D[bh_p, j] * D[bw_p, k])
    W16[k][(q,j), 64:128] = fp16_lo residual * 512

so per (c, gh): 8 fp16 matmuls accumulate hi products into PSUM rows
0:64 and 512*lo products into rows 64:128, then 8 fp8 matmuls (W*4 @
r*128 = 512*W@r) accumulate the residual correction into the SAME rows
64:128.  96 matmuls of 512 cols total; PE time ~21 us, well under the
~24 us the 9.3 MB input stream needs at the ~400 GB/s the sync-queue
DMA sustains.  The parity chain is 4 ops: Scalar stages the lo half
out of PSUM (*1/512, exact), Vector folds (hi + lo), adds 1.5*2^23
(pinning the exponent so the f32 mantissa LSB IS the parity of the
RNE-rounded coefficient), a bitwise AND with 1 extracts it, and a
convert-copy emits numeric bf16 0/1 (halving the output stream).

All DMA rides the sync queue in need-order (gh-major, so each
half-channel's accumulation closes and drains while the next half
streams; the last 256 KB piece gates only 4 matmuls + one short
chain).  A leading 256 KB piece + the weights let the PE start as
early as possible.

General fallback (arbitrary indices): device computes the full 64-plane
parity table per image; host gathers bits and applies the b mask.
"""

import sys

if "/opt/trn_rl_repo" not in sys.path:
    sys.path.insert(0, "/opt/trn_rl_repo")

import numpy as np
import ml_dtypes

BS = 8
B, C, H, W = 8, 3, 1024, 1024
NBH, NBW = H // BS, W // BS
POS = np.array([[1, 2], [2, 1], [2, 2], [3, 1]], dtype=np.int32)
NPOS = 4
SEG = C * NBH * NBW * NPOS  # bits per batch element = 196608
NUM_BITS = B * SEG
MAGIC = float(np.float32(8388608.0))  # 2^23 (general path, abs first)
M15 = float(np.float32(12582912.0))  # 1.5*2^23: RNE for signed values
IP = [0, 1, 1, 2]  # i' = bh-1 per p
LP = [2, 1, 2, 1]  # l  = bw   per p

_CACHE = {}


def _split_sync_waits(nc):
    """The staged walrus build accepts at most ONE sync wait per
    instruction, but Tile's wait-assignment freely attaches several.
    Hoist all but the last wait of each instruction onto same-engine
    NoOps inserted directly before it (engines execute their stream in
    order, so the semantics are identical)."""
    from concourse import mybir

    if getattr(nc, "_sync_waits_split", False):
        return
    nc._sync_waits_split = True
    counter = 0
    for bb in nc.m.functions[0].blocks:
        out = []
        changed = False
        for inst in bb.instructions:
            si = inst.sync_info
            waits = list(si.on_wait) if si is not None else []
            if len(waits) > 1:
                for w in waits[:-1]:
                    nop = mybir.InstNoOp(
                        name=f"I-splitw-{counter}", ins=[], outs=[])
                    counter += 1
                    nop.engine = inst.engine
                    nop.sync_info = mybir.SyncInfo(on_update=[], on_wait=[w])
                    out.append(nop)
                si.on_wait = waits[-1:]
                changed = True
            out.append(inst)
        if changed:
            bb.instructions = out
    return


def _dct_matrix_f32() -> np.ndarray:
    k = np.arange(BS)[:, None].astype(np.float64)
    m = np.arange(BS)[None, :].astype(np.float64)
    D = np.cos(np.pi * (2.0 * m + 1.0) * k / (2.0 * BS)) * np.sqrt(2.0 / BS)
    D[0, :] = np.sqrt(1.0 / BS)
    return D.astype(np.float32)


def _canonical_indices():
    b, c, nh, nw, p = np.meshgrid(
        np.arange(B), np.arange(C), np.arange(NBH), np.arange(NBW),
        np.arange(NPOS), indexing="ij")
    return {
        "b_idx": b.reshape(-1).astype(np.int32),
        "c_idx": c.reshape(-1).astype(np.int32),
        "nh_idx": nh.reshape(-1).astype(np.int32),
        "nw_idx": nw.reshape(-1).astype(np.int32),
        "bh_idx": POS[p.reshape(-1), 0].astype(np.int32),
        "bw_idx": POS[p.reshape(-1), 1].astype(np.int32),
    }


def _is_canonical(b_idx, c_idx, nh_idx, nw_idx, bh_idx, bw_idx) -> bool:
    if b_idx.shape[0] != NUM_BITS:
        return False
    canon = _CACHE.setdefault("canon", _canonical_indices())
    got = {"b_idx": b_idx, "c_idx": c_idx, "nh_idx": nh_idx,
           "nw_idx": nw_idx, "bh_idx": bh_idx, "bw_idx": bw_idx}
    return all(np.array_equal(np.asarray(got[k]), canon[k]) for k in canon)


def _build_w_fast():
    """w16 [128, 1024] fp16: [Wa16|Wb16] hi/lo planes per k;
    w8 [128, 512] fp8e4m3: W*4 per k (the *4 keeps every entry out of
    the e4m3 subnormal range; the matching 1/512 lives in the chain)."""
    D = _dct_matrix_f32()
    Wt = np.zeros((128, 8, 64), dtype=np.float32)
    for k in range(8):
        for q in range(16):
            for j in range(8):
                for p in range(4):
                    Wt[q * 8 + j, k, q * 4 + p] = (
                        D[1 + IP[p], j] * D[LP[p], k])
    Wa = Wt.astype(np.float16)
    # lo plane pre-scaled by 512 so the fp8 residual matmuls (4W @ 128r
    # = 512*W@r) can accumulate into the SAME PSUM rows; the chain
    # divides the whole lo half by 512 (exact).
    Wb = ((Wt - Wa.astype(np.float32)) * 512.0).astype(np.float16)
    w16 = np.ascontiguousarray(
        np.concatenate([Wa, Wb], axis=2)).reshape(128, 1024)
    w8 = np.ascontiguousarray(
        (Wt * 4.0).astype(ml_dtypes.float8_e4m3)).reshape(128, 512)
    return w16, w8


def build_fast_nc():
    """Per-core program: x16 [3, 128, 8192] fp16 + r8 [3, 128, 8192]
    fp8 (residual*128) -> po [3, 64, 1024] bf16 parity bits,
    po[c][q*4+p][g*128+nw]."""
    import concourse.bass as bass
    import concourse.tile as tile
    from concourse import mybir

    f32 = mybir.dt.float32
    f16 = mybir.dt.float16
    f8 = mybir.dt.float8e4
    bf = mybir.dt.bfloat16
    nc = bass.Bass()
    x16 = nc.dram_tensor("x16", [C, 128, 8192], f16, kind="ExternalInput")
    r8 = nc.dram_tensor("r8", [C, 128, 8192], f8, kind="ExternalInput")
    w16 = nc.dram_tensor("w16", [128, 1024], f16, kind="ExternalInput")
    w8 = nc.dram_tensor("w8", [128, 512], f8, kind="ExternalInput")
    po = nc.dram_tensor("po", [C, 64, 1024], bf, kind="ExternalOutput")

    add = mybir.AluOpType.add
    mult = mybir.AluOpType.mult

    band = mybir.AluOpType.bitwise_and
    i32 = mybir.dt.int32

    def chain(acc, tmp, par_dst, one_col):
        """parity of RNE(acc[0:64] + acc[64:128]/512) -> par_dst
        (bf16 0/1; host maps nonzero->1 for robustness).  acc
        rows 0:64 hi-plane, rows 64:128 hold 512*(lo-plane + W@r).
        s + M15 = M15 + rne(s) exactly (1.5*2^23 pins the exponent for
        |s| < 2^22), so the f32 mantissa LSB IS the parity; an int32
        bitwise-and with 1 extracts it.  Only one PSUM operand per
        instruction is allowed, hence the staging Copy (1/512 exact)."""
        n = par_dst.free_size()
        s = tmp.tile([64, n], f32, tag="s", bufs=3)
        u = tmp.tile([64, n], f32, tag="u", bufs=3)
        nc.scalar.activation(
            out=u[:], in_=acc[64:128, :],
            func=mybir.ActivationFunctionType.Copy, scale=1.0 / 512.0)
        nc.vector.tensor_tensor(
            out=s[:], in0=acc[0:64, :], in1=u[:], op=add)
        nc.vector.tensor_scalar(out=u[:], in0=s[:], scalar1=M15,
                                scalar2=None, op0=add)
        nc.vector.tensor_scalar(out=s[:].bitcast(i32),
                                in0=u[:].bitcast(i32), scalar1=one_col,
                                scalar2=None, op0=band)
        # int32 0/1 -> numeric bf16 0.0/1.0: halves the po stream (the
        # host still maps nonzero->1, so truncating-bitcast semantics
        # of the copy would be tolerated too)
        nc.vector.tensor_copy(out=par_dst, in_=s[:].bitcast(i32))

    with tile.TileContext(nc) as tc:
        with (
            tc.tile_pool(name="sb", bufs=1) as sb,
            tc.tile_pool(name="ps", bufs=8, space="PSUM") as ps,
        ):
            wt16 = sb.tile([128, 1024], f16, tag="w16")
            wt8 = sb.tile([128, 512], f8, tag="w8")
            ones = sb.tile([128, 1], i32, tag="ones")
            nc.vector.memset(ones[:], 1)
            one_col = ones[0:64, :]

            # Input stream on the sync queue in need-order, gh-major so
            # each half-channel's accumulation closes (and its chain +
            # output drain) while the next half streams.
            xtiles = []
            for c in range(C):
                xt = sb.tile([128, 8192], f16, tag="x16", bufs=3,
                             name=f"xt{c}")
                rt = sb.tile([128, 8192], f8, tag="r8", bufs=3,
                             name=f"rt{c}")
                xtiles.append((xt, rt))
                for gh in range(2):
                    if c == 0 and gh == 0:
                        # weights first (their small 2KB-line
                        # descriptors ride the issue-serialized first
                        # us); all x16 pieces are ONE 1 MB 8KB-line
                        # descriptor per half-channel — small-descriptor
                        # heads underfeed the 16 DMA engines (~240 GB/s
                        # through their ramp)
                        nc.sync.dma_start(out=wt16[:], in_=w16[:, :])
                        nc.sync.dma_start(out=wt8[:], in_=w8[:, :])
                    sl = slice(gh * 4096, (gh + 1) * 4096)
                    nc.sync.dma_start(out=xt[:, sl], in_=x16[c][:, sl])
                    for j in range(2):  # 2048-col fp8 pieces (256 KB):
                        # finer pieces are ISSUE-bound (~530 ns/descriptor
                        # on the sync engine vs ~640 ns transfer)
                        sl = slice((gh * 2 + j) * 2048,
                                   (gh * 2 + j + 1) * 2048)
                        nc.sync.dma_start(out=rt[:, sl], in_=r8[c][:, sl])

            for c in range(C):
                xt, rt = xtiles[c]
                par_c = sb.tile([64, 1024], bf, tag="par", bufs=3)
                for gh in range(2):
                    if c == 2 and gh == 1:
                        break  # handled below with a split tail
                    acc = ps.tile([128, 512], f32, tag="ps",
                                  name=f"acc{c}{gh}", bufs=6)
                    for k in range(8):
                        base = gh * 4096 + k * 512
                        nc.tensor.matmul(
                            out=acc[:, :],
                            lhsT=wt16[:, k * 128:(k + 1) * 128],
                            rhs=xt[:, base:base + 512],
                            start=(k == 0), stop=False)
                    for k in range(8):
                        base = gh * 4096 + k * 512
                        nc.tensor.matmul(
                            out=acc[64:128, :],
                            lhsT=wt8[:, k * 64:(k + 1) * 64],
                            rhs=rt[:, base:base + 512],
                            start=False, stop=(k == 7))
                    chain(acc[:], sb, par_c[:, gh * 512:(gh + 1) * 512],
                          one_col)
                    nc.sync.dma_start(
                        out=po[c][:, gh * 512:(gh + 1) * 512],
                        in_=par_c[:, gh * 512:(gh + 1) * 512])

            # The very last half-channel (c2, gh1) runs as TWO column-
            # half accumulation groups so that after the final 256 KB
            # r8 piece lands, half A closes in 4 short matmuls and its
            # chain overlaps half B's matmuls — ~1 us off the tail.
            xt, rt = xtiles[2]
            accs = [ps.tile([128, 256], f32, tag="ps2",
                            name=f"accL{h}", bufs=2) for h in range(2)]
            for k in range(8):
                base = 4096 + k * 512
                for h in range(2):
                    nc.tensor.matmul(
                        out=accs[h][:, :],
                        lhsT=wt16[:, k * 128:(k + 1) * 128],
                        rhs=xt[:, base + h * 256:base + (h + 1) * 256],
                        start=(k == 0), stop=False)
            for k in range(4):
                base = 4096 + k * 512
                for h in range(2):
                    nc.tensor.matmul(
                        out=accs[h][64:128, :],
                        lhsT=wt8[:, k * 64:(k + 1) * 64],
                        rhs=rt[:, base + h * 256:base + (h + 1) * 256],
                        start=False, stop=False)
            for h in range(2):  # gated by the final piece: h-major
                for k in range(4, 8):
                    base = 4096 + k * 512
                    nc.tensor.matmul(
                        out=accs[h][64:128, :],
                        lhsT=wt8[:, k * 64:(k + 1) * 64],
                        rhs=rt[:, base + h * 256:base + (h + 1) * 256],
                        start=False, stop=(k == 7))
            for h in range(2):
                sl = slice(512 + h * 256, 512 + (h + 1) * 256)
                chain(accs[h][:], sb, par_c[:, sl], one_col)
                nc.sync.dma_start(out=po[2][:, sl], in_=par_c[:, sl])
    return nc


def _prep_core(x):
    """x [3,1024,1024] f32 -> (x16 fp16, r8 fp8e4m3 of residual*128) in
    the (q,j)-partition, (gh, k, g', nw)-free layout (g = gh*4+g')."""
    x16 = x.astype(np.float16)
    r = x - x16.astype(np.float32)
    r8 = (r * 128.0).astype(ml_dtypes.float8_e4m3)

    def rl(a):
        # (c, gh, g', q, j, nw, k) -> (c, q, j, gh, k, g', nw)
        return np.ascontiguousarray(
            a.reshape(C, 2, 4, 16, 8, 128, 8).transpose(0, 3, 4, 1, 6, 2, 5)
        ).reshape(C, 128, 8192)

    return rl(x16), rl(r8)


def _build_consts_general():
    D = _dct_matrix_f32()
    BR8 = np.zeros((128, 128), dtype=np.float32)
    for nhl in range(16):
        BR8[nhl * 8:(nhl + 1) * 8, nhl * 8:(nhl + 1) * 8] = D.T  # [j, i]
    BC8 = np.zeros((128, 128), dtype=np.float32)
    for l in range(8):
        for nwl in range(16):
            BC8[nwl * 8:(nwl + 1) * 8, l * 16 + nwl] = D[l, :]
    return BR8, BC8


def _parity_ops_general(nc, pk, hk):
    """pk holds |coeff| >= 0; parity via 2^23 magic (each step its own
    instruction so every intermediate is rounded f32)."""
    from concourse import mybir

    ts = nc.vector.tensor_scalar
    add, sub, mult = (mybir.AluOpType.add, mybir.AluOpType.subtract,
                      mybir.AluOpType.mult)
    ts(out=pk[:], in0=pk[:], scalar1=MAGIC, scalar2=None, op0=add)
    ts(out=pk[:], in0=pk[:], scalar1=MAGIC, scalar2=None, op0=sub)
    ts(out=hk[:], in0=pk[:], scalar1=0.5, scalar2=None, op0=mult)
    ts(out=pk[:], in0=hk[:], scalar1=MAGIC, scalar2=None, op0=add)
    ts(out=pk[:], in0=pk[:], scalar1=MAGIC, scalar2=None, op0=sub)
    nc.vector.tensor_tensor(out=pk[:], in0=hk[:], in1=pk[:], op=sub)
    nc.scalar.activation(
        out=pk[:], in_=pk[:], func=mybir.ActivationFunctionType.Abs,
        scale=2.0)


def build_general_nc(nstrip=C * (H // 128)):
    """Per-core program: full 64-plane parity table (see baseline)."""
    import concourse.bass as bass
    import concourse.tile as tile
    from concourse import mybir

    f32 = mybir.dt.float32
    nc = bass.Bass()
    x = nc.dram_tensor("x", [C, H, W], f32, kind="ExternalInput")
    br = nc.dram_tensor("br", [128, 128], f32, kind="ExternalInput")
    bc = nc.dram_tensor("bc", [128, 128], f32, kind="ExternalInput")
    o = nc.dram_tensor("o", [nstrip, 128, 1024], f32, kind="ExternalOutput")

    with tile.TileContext(nc) as tc:
        with (
            tc.tile_pool(name="consts", bufs=1) as consts,
            tc.tile_pool(name="xs", bufs=2) as xpool,
            tc.tile_pool(name="ysb", bufs=2) as ypool,
            tc.tile_pool(name="pk", bufs=2) as pkpool,
            tc.tile_pool(name="yp", bufs=4, space="PSUM") as yppool,
            tc.tile_pool(name="fp", bufs=4, space="PSUM") as fppool,
        ):
            brt = consts.tile([128, 128], f32)
            nc.sync.dma_start(out=brt[:], in_=br[:, :])
            bct = consts.tile([128, 128], f32)
            nc.sync.dma_start(out=bct[:], in_=bc[:, :])

            for s in range(nstrip):
                c, hg = divmod(s, H // 128)
                xs = xpool.tile([128, 1024], f32, tag="xs")
                nc.sync.dma_start(
                    out=xs[:], in_=x[c, hg * 128:(hg + 1) * 128, :])
                ysb = ypool.tile([128, 1024], f32, tag="ysb")
                for wc in range(8):
                    yp = yppool.tile([128, 128], f32, tag="yp")
                    nc.tensor.matmul(
                        out=yp[:],
                        lhsT=xs[:, wc * 128:(wc + 1) * 128],
                        rhs=brt[:],
                        start=True, stop=True)
                    nc.vector.tensor_copy(
                        out=ysb[:, wc * 128:(wc + 1) * 128], in_=yp[:])
                pk = pkpool.tile([128, 1024], f32, tag="pk")
                hk = pkpool.tile([128, 1024], f32, tag="hk")
                for wc in range(8):
                    fp = fppool.tile([128, 128], f32, tag="fp")
                    nc.tensor.matmul(
                        out=fp[:],
                        lhsT=bct[:],
                        rhs=ysb[:, wc * 128:(wc + 1) * 128],
                        start=True, stop=True)
                    nc.scalar.activation(
                        out=pk[:, wc * 128:(wc + 1) * 128], in_=fp[:],
                        func=mybir.ActivationFunctionType.Abs)
                _parity_ops_general(nc, pk, hk)
                nc.sync.dma_start(out=o[s], in_=pk[:])
    return nc


def _run_spmd(nc, in_maps, trace=False):
    from concourse.bass_utils import run_bass_kernel_spmd

    _split_sync_waits(nc)

    res = run_bass_kernel_spmd(
        nc, in_maps, core_ids=list(range(B)), trace=trace)
    _CACHE["last_results"] = res
    return res.results


def _fast_path(stego, trace=False):
    key = "fast_nc5"
    if key not in _CACHE:
        _CACHE[key] = build_fast_nc()
    nc = _CACHE[key]
    w16, w8 = _CACHE.setdefault("consts_fast5", _build_w_fast())
    in_maps = []
    for b in range(B):
        x16, r8 = _prep_core(stego[b])
        in_maps.append({"x16": x16, "r8": r8, "w16": w16, "w8": w8})
    results = _run_spmd(nc, in_maps, trace=trace)
    out = np.zeros((B, NUM_BITS), dtype=np.float32)
    for b in range(B):
        po = (results[b]["po"].view(np.uint16) != 0).astype(
            np.float32)  # [3, 64, 1024]
        seg = np.ascontiguousarray(
            po.reshape(C, 16, 4, 8, 128).transpose(0, 3, 1, 4, 2)
        ).reshape(-1)
        out[b, b * SEG:(b + 1) * SEG] = seg
    return out


def _general_path(stego, b_idx, c_idx, nh_idx, nw_idx, bh_idx, bw_idx,
                  trace=False):
    key = "general_nc"
    if key not in _CACHE:
        _CACHE[key] = build_general_nc()
    nc = _CACHE[key]
    BR8, BC8 = _CACHE.setdefault("consts_general", _build_consts_general())
    in_maps = [
        {"x": np.ascontiguousarray(stego[b]), "br": BR8, "bc": BC8}
        for b in range(B)
    ]
    results = _run_spmd(nc, in_maps, trace=trace)

    b_idx = np.asarray(b_idx).astype(np.int64)
    c_idx = np.asarray(c_idx).astype(np.int64)
    nh_idx = np.asarray(nh_idx).astype(np.int64)
    nw_idx = np.asarray(nw_idx).astype(np.int64)
    bh_idx = np.asarray(bh_idx).astype(np.int64)
    bw_idx = np.asarray(bw_idx).astype(np.int64)
    num_bits = b_idx.shape[0]

    # table[s=(c,hg), l*16+nwl, wc*128 + nhl*8 + i]
    s = c_idx * 8 + nh_idx // 16
    part = bw_idx * 16 + nw_idx % 16
    free = (nw_idx // 16) * 128 + (nh_idx % 16) * 8 + bh_idx
    flat = (s * 128 + part) * 1024 + free

    out = np.zeros((B, num_bits), dtype=np.float32)
    cols = np.arange(num_bits)
    for b in range(B):
        tb = results[b]["o"].reshape(-1)
        mask = b_idx == b
        out[b, cols[mask]] = tb[flat[mask]]
    return out


def kernel(stego, b_idx, c_idx, nh_idx, nw_idx, bh_idx, bw_idx):
    stego = np.ascontiguousarray(np.asarray(stego, dtype=np.float32))
    import os
    trace = os.environ.get("BASS_TRACE", "") not in ("", "0")
    if _is_canonical(b_idx, c_idx, nh_idx, nw_idx, bh_idx, bw_idx):
        return _fast_path(stego, trace=trace)
    return _general_path(
        stego, b_idx, c_idx, nh_idx, nw_idx, bh_idx, bw_idx, trace=trace)


# revision 59
# speedup vs baseline: 1.0435x; 1.0435x over previous
"""Trainium2 Bass kernel for nn_DCTExtractor.

Reference computation:
  - stego [8, 3, 1024, 1024] f32; per 8x8 block 2D DCT-II (D @ X @ D^T).
  - bits[i] = abs(round_half_even(dct[b,c,nh,nw,bh,bw])) % 2 for 1572864
    index tuples.
  - out [8, num_bits]: out[b_idx[i], i] = bits[i]; other rows 0.

Sharding: data-parallel over batch b across the 8 NeuronCores; core b
processes image b and produces output row b.

Fast path (canonical meshgrid indices): each needed coefficient is a
Frobenius inner product <X_block, D_bh (x) D_bw>.  The kernel is
HBM-bandwidth bound, so the host ships each image as 3 bytes/pixel:
an fp16 hi plane and an fp8e4m3 plane of the residual*128, both in a
layout with the two within-block axes on partitions:

    x16/r8[c, (q,j) 128, (gh, k, g', nw) 8192]   nh = (gh*4+g')*16 + q,
                                                 w = nw*8 + k

The fp16 stationary packs hi/lo weight planes side by side (the lo
plane pre-scaled by 512)

    W16[k][(q,j), 0:64]   = fp16_hi(D[bh_p, j] * D[bw_p, k])
    W16[k][(q,j), 64:128] = fp16_lo residual * 512

so per (c, gh): 8 fp16 matmuls accumulate hi products into PSUM rows
0:64 and 512*lo products into rows 64:128, then 8 fp8 matmuls (W*4 @
r*128 = 512*W@r) accumulate the residual correction into the SAME rows
64:128.  96 matmuls of 512 cols total; PE time ~21 us, well under the
~24 us the 9.3 MB input stream needs at the ~400 GB/s the sync-queue
DMA sustains.  The parity chain is 4 ops: Scalar stages the lo half
out of PSUM (*1/512, exact), Vector folds (hi + lo), adds 1.5*2^23
(pinning the exponent so the f32 mantissa LSB IS the parity of the
RNE-rounded coefficient), a bitwise AND with 1 extracts it, and a
convert-copy emits numeric bf16 0/1 (halving the output stream).

All DMA rides the sync queue in need-order (gh-major, so each
half-channel's accumulation closes and drains while the next half
streams; the last 256 KB piece gates only 4 matmuls + one short
chain).  A leading 256 KB piece + the weights let the PE start as
early as possible.

General fallback (arbitrary indices): device computes the full 64-plane
parity table per image; host gathers bits and applies the b mask.
"""

import sys

if "/opt/trn_rl_repo" not in sys.path:
    sys.path.insert(0, "/opt/trn_rl_repo")

import numpy as np
import ml_dtypes

BS = 8
B, C, H, W = 8, 3, 1024, 1024
NBH, NBW = H // BS, W // BS
POS = np.array([[1, 2], [2, 1], [2, 2], [3, 1]], dtype=np.int32)
NPOS = 4
SEG = C * NBH * NBW * NPOS  # bits per batch element = 196608
NUM_BITS = B * SEG
MAGIC = float(np.float32(8388608.0))  # 2^23 (general path, abs first)
M15 = float(np.float32(12582912.0))  # 1.5*2^23: RNE for signed values
IP = [0, 1, 1, 2]  # i' = bh-1 per p
LP = [2, 1, 2, 1]  # l  = bw   per p

_CACHE = {}


def _split_sync_waits(nc):
    """The staged walrus build accepts at most ONE sync wait per
    instruction, but Tile's wait-assignment freely attaches several.
    Hoist all but the last wait of each instruction onto same-engine
    NoOps inserted directly before it (engines execute their stream in
    order, so the semantics are identical)."""
    from concourse import mybir

    if getattr(nc, "_sync_waits_split", False):
        return
    nc._sync_waits_split = True
    counter = 0
    for bb in nc.m.functions[0].blocks:
        out = []
        changed = False
        for inst in bb.instructions:
            si = inst.sync_info
            waits = list(si.on_wait) if si is not None else []
            if len(waits) > 1:
                for w in waits[:-1]:
                    nop = mybir.InstNoOp(
                        name=f"I-splitw-{counter}", ins=[], outs=[])
                    counter += 1
                    nop.engine = inst.engine
                    nop.sync_info = mybir.SyncInfo(on_update=[], on_wait=[w])
                    out.append(nop)
                si.on_wait = waits[-1:]
                changed = True
            out.append(inst)
        if changed:
            bb.instructions = out
    return


def _dct_matrix_f32() -> np.ndarray:
    k = np.arange(BS)[:, None].astype(np.float64)
    m = np.arange(BS)[None, :].astype(np.float64)
    D = np.cos(np.pi * (2.0 * m + 1.0) * k / (2.0 * BS)) * np.sqrt(2.0 / BS)
    D[0, :] = np.sqrt(1.0 / BS)
    return D.astype(np.float32)


def _canonical_indices():
    b, c, nh, nw, p = np.meshgrid(
        np.arange(B), np.arange(C), np.arange(NBH), np.arange(NBW),
        np.arange(NPOS), indexing="ij")
    return {
        "b_idx": b.reshape(-1).astype(np.int32),
        "c_idx": c.reshape(-1).astype(np.int32),
        "nh_idx": nh.reshape(-1).astype(np.int32),
        "nw_idx": nw.reshape(-1).astype(np.int32),
        "bh_idx": POS[p.reshape(-1), 0].astype(np.int32),
        "bw_idx": POS[p.reshape(-1), 1].astype(np.int32),
    }


def _is_canonical(b_idx, c_idx, nh_idx, nw_idx, bh_idx, bw_idx) -> bool:
    if b_idx.shape[0] != NUM_BITS:
        return False
    canon = _CACHE.setdefault("canon", _canonical_indices())
    got = {"b_idx": b_idx, "c_idx": c_idx, "nh_idx": nh_idx,
           "nw_idx": nw_idx, "bh_idx": bh_idx, "bw_idx": bw_idx}
    return all(np.array_equal(np.asarray(got[k]), canon[k]) for k in canon)


def _build_w_fast():
    """w16 [128, 1024] fp16: [Wa16|Wb16] hi/lo planes per k;
    w8 [128, 512] fp8e4m3: W*4 per k (the *4 keeps every entry out of
    the e4m3 subnormal range; the matching 1/512 lives in the chain)."""
    D = _dct_matrix_f32()
    Wt = np.zeros((128, 8, 64), dtype=np.float32)
    for k in range(8):
        for q in range(16):
            for j in range(8):
                for p in range(4):
                    Wt[q * 8 + j, k, q * 4 + p] = (
                        D[1 + IP[p], j] * D[LP[p], k])
    Wa = Wt.astype(np.float16)
    # lo plane pre-scaled by 512 so the fp8 residual matmuls (4W @ 128r
    # = 512*W@r) can accumulate into the SAME PSUM rows; the chain
    # divides the whole lo half by 512 (exact).
    Wb = ((Wt - Wa.astype(np.float32)) * 512.0).astype(np.float16)
    w16 = np.ascontiguousarray(
        np.concatenate([Wa, Wb], axis=2)).reshape(128, 1024)
    w8 = np.ascontiguousarray(
        (Wt * 4.0).astype(ml_dtypes.float8_e4m3)).reshape(128, 512)
    return w16, w8


def build_fast_nc():
    """Per-core program: x16 [3, 128, 8192] fp16 + r8 [3, 128, 8192]
    fp8 (residual*128) -> po [3, 64, 1024] bf16 parity bits,
    po[c][q*4+p][g*128+nw]."""
    import concourse.bass as bass
    import concourse.tile as tile
    from concourse import mybir

    f32 = mybir.dt.float32
    f16 = mybir.dt.float16
    f8 = mybir.dt.float8e4
    bf = mybir.dt.bfloat16
    nc = bass.Bass()
    x16 = nc.dram_tensor("x16", [C, 128, 8192], f16, kind="ExternalInput")
    r8 = nc.dram_tensor("r8", [C, 128, 8192], f8, kind="ExternalInput")
    w16 = nc.dram_tensor("w16", [128, 1024], f16, kind="ExternalInput")
    w8 = nc.dram_tensor("w8", [128, 512], f8, kind="ExternalInput")
    po = nc.dram_tensor("po", [C, 64, 1024], bf, kind="ExternalOutput")

    add = mybir.AluOpType.add
    mult = mybir.AluOpType.mult

    band = mybir.AluOpType.bitwise_and
    i32 = mybir.dt.int32

    def chain(acc, tmp, par_dst, one_col):
        """parity of RNE(acc[0:64] + acc[64:128]/512) -> par_dst
        (bf16 0/1; host maps nonzero->1 for robustness).  acc
        rows 0:64 hi-plane, rows 64:128 hold 512*(lo-plane + W@r).
        s + M15 = M15 + rne(s) exactly (1.5*2^23 pins the exponent for
        |s| < 2^22), so the f32 mantissa LSB IS the parity; an int32
        bitwise-and with 1 extracts it.  Only one PSUM operand per
        instruction is allowed, hence the staging Copy (1/512 exact)."""
        n = par_dst.free_size()
        s = tmp.tile([64, n], f32, tag="s", bufs=3)
        u = tmp.tile([64, n], f32, tag="u", bufs=3)
        nc.scalar.activation(
            out=u[:], in_=acc[64:128, :],
            func=mybir.ActivationFunctionType.Copy, scale=1.0 / 512.0)
        nc.vector.tensor_tensor(
            out=s[:], in0=acc[0:64, :], in1=u[:], op=add)
        nc.vector.tensor_scalar(out=u[:], in0=s[:], scalar1=M15,
                                scalar2=None, op0=add)
        nc.vector.tensor_scalar(out=s[:].bitcast(i32),
                                in0=u[:].bitcast(i32), scalar1=one_col,
                                scalar2=None, op0=band)
        # int32 0/1 -> numeric bf16 0.0/1.0: halves the po stream (the
        # host still maps nonzero->1, so truncating-bitcast semantics
        # of the copy would be tolerated too)
        nc.vector.tensor_copy(out=par_dst, in_=s[:].bitcast(i32))

    with tile.TileContext(nc) as tc:
        with (
            tc.tile_pool(name="sb", bufs=1) as sb,
            tc.tile_pool(name="ps", bufs=8, space="PSUM") as ps,
        ):
            wt16 = sb.tile([128, 1024], f16, tag="w16")
            wt8 = sb.tile([128, 512], f8, tag="w8")
            ones = sb.tile([128, 1], i32, tag="ones")
            nc.vector.memset(ones[:], 1)
            one_col = ones[0:64, :]

            # Input stream on the sync queue in need-order, gh-major so
            # each half-channel's accumulation closes (and its chain +
            # output drain) while the next half streams.
            xtiles = []
            for c in range(C):
                xt = sb.tile([128, 8192], f16, tag="x16", bufs=3,
                             name=f"xt{c}")
                rt = sb.tile([128, 8192], f8, tag="r8", bufs=3,
                             name=f"rt{c}")
                xtiles.append((xt, rt))
                for gh in range(2):
                    for j in range(2):  # 2048-col fp16 pieces (512 KB)
                        sl = slice((gh * 2 + j) * 2048,
                                   (gh * 2 + j + 1) * 2048)
                        if c == 0 and gh == 0:
                            if j == 0:
                                # weights first (their small 2KB-line
                                # descriptors ride the issue-serialized
                                # first us), then ONE large 8KB-line
                                # piece to feed the 16 DMA engines
                                # through their ramp (a small-descriptor
                                # head measured ~240 GB/s)
                                nc.sync.dma_start(out=wt16[:],
                                                  in_=w16[:, :])
                                nc.sync.dma_start(out=wt8[:],
                                                  in_=w8[:, :])
                                nc.sync.dma_start(out=xt[:, 0:4096],
                                                  in_=x16[c][:, 0:4096])
                            continue
                        nc.sync.dma_start(out=xt[:, sl], in_=x16[c][:, sl])
                    for j in range(2):  # 2048-col fp8 pieces (256 KB):
                        # finer pieces are ISSUE-bound (~530 ns/descriptor
                        # on the sync engine vs ~640 ns transfer)
                        sl = slice((gh * 2 + j) * 2048,
                                   (gh * 2 + j + 1) * 2048)
                        nc.sync.dma_start(out=rt[:, sl], in_=r8[c][:, sl])

            for c in range(C):
                xt, rt = xtiles[c]
                par_c = sb.tile([64, 1024], bf, tag="par", bufs=3)
                for gh in range(2):
                    if c == 2 and gh == 1:
                        break  # handled below with a split tail
                    acc = ps.tile([128, 512], f32, tag="ps",
                                  name=f"acc{c}{gh}", bufs=6)
                    for k in range(8):
                        base = gh * 4096 + k * 512
                        nc.tensor.matmul(
                            out=acc[:, :],
                            lhsT=wt16[:, k * 128:(k + 1) * 128],
                            rhs=xt[:, base:base + 512],
                            start=(k == 0), stop=False)
                    for k in range(8):
                        base = gh * 4096 + k * 512
                        nc.tensor.matmul(
                            out=acc[64:128, :],
                            lhsT=wt8[:, k * 64:(k + 1) * 64],
                            rhs=rt[:, base:base + 512],
                            start=False, stop=(k == 7))
                    chain(acc[:], sb, par_c[:, gh * 512:(gh + 1) * 512],
                          one_col)
                    nc.sync.dma_start(
                        out=po[c][:, gh * 512:(gh + 1) * 512],
                        in_=par_c[:, gh * 512:(gh + 1) * 512])

            # The very last half-channel (c2, gh1) runs as TWO column-
            # half accumulation groups so that after the final 256 KB
            # r8 piece lands, half A closes in 4 short matmuls and its
            # chain overlaps half B's matmuls — ~1 us off the tail.
            xt, rt = xtiles[2]
            accs = [ps.tile([128, 256], f32, tag="ps2",
                            name=f"accL{h}", bufs=2) for h in range(2)]
            for k in range(8):
                base = 4096 + k * 512
                for h in range(2):
                    nc.tensor.matmul(
                        out=accs[h][:, :],
                        lhsT=wt16[:, k * 128:(k + 1) * 128],
                        rhs=xt[:, base + h * 256:base + (h + 1) * 256],
                        start=(k == 0), stop=False)
            for k in range(4):
                base = 4096 + k * 512
                for h in range(2):
                    nc.tensor.matmul(
                        out=accs[h][64:128, :],
                        lhsT=wt8[:, k * 64:(k + 1) * 64],
                        rhs=rt[:, base + h * 256:base + (h + 1) * 256],
                        start=False, stop=False)
            for h in range(2):  # gated by the final piece: h-major
                for k in range(4, 8):
                    base = 4096 + k * 512
                    nc.tensor.matmul(
                        out=accs[h][64:128, :],
                        lhsT=wt8[:, k * 64:(k + 1) * 64],
                        rhs=rt[:, base + h * 256:base + (h + 1) * 256],
                        start=False, stop=(k == 7))
            for h in range(2):
                sl = slice(512 + h * 256, 512 + (h + 1) * 256)
                chain(accs[h][:], sb, par_c[:, sl], one_col)
                nc.sync.dma_start(out=po[2][:, sl], in_=par_c[:, sl])
    return nc


def _prep_core(x):
    """x [3,1024,1024] f32 -> (x16 fp16, r8 fp8e4m3 of residual*128) in
    the (q,j)-partition, (gh, k, g', nw)-free layout (g = gh*4+g')."""
    x16 = x.astype(np.float16)
    r = x - x16.astype(np.float32)
    r8 = (r * 128.0).astype(ml_dtypes.float8_e4m3)

    def rl(a):
        # (c, gh, g', q, j, nw, k) -> (c, q, j, gh, k, g', nw)
        return np.ascontiguousarray(
            a.reshape(C, 2, 4, 16, 8, 128, 8).transpose(0, 3, 4, 1, 6, 2, 5)
        ).reshape(C, 128, 8192)

    return rl(x16), rl(r8)


def _build_consts_general():
    D = _dct_matrix_f32()
    BR8 = np.zeros((128, 128), dtype=np.float32)
    for nhl in range(16):
        BR8[nhl * 8:(nhl + 1) * 8, nhl * 8:(nhl + 1) * 8] = D.T  # [j, i]
    BC8 = np.zeros((128, 128), dtype=np.float32)
    for l in range(8):
        for nwl in range(16):
            BC8[nwl * 8:(nwl + 1) * 8, l * 16 + nwl] = D[l, :]
    return BR8, BC8


def _parity_ops_general(nc, pk, hk):
    """pk holds |coeff| >= 0; parity via 2^23 magic (each step its own
    instruction so every intermediate is rounded f32)."""
    from concourse import mybir

    ts = nc.vector.tensor_scalar
    add, sub, mult = (mybir.AluOpType.add, mybir.AluOpType.subtract,
                      mybir.AluOpType.mult)
    ts(out=pk[:], in0=pk[:], scalar1=MAGIC, scalar2=None, op0=add)
    ts(out=pk[:], in0=pk[:], scalar1=MAGIC, scalar2=None, op0=sub)
    ts(out=hk[:], in0=pk[:], scalar1=0.5, scalar2=None, op0=mult)
    ts(out=pk[:], in0=hk[:], scalar1=MAGIC, scalar2=None, op0=add)
    ts(out=pk[:], in0=pk[:], scalar1=MAGIC, scalar2=None, op0=sub)
    nc.vector.tensor_tensor(out=pk[:], in0=hk[:], in1=pk[:], op=sub)
    nc.scalar.activation(
        out=pk[:], in_=pk[:], func=mybir.ActivationFunctionType.Abs,
        scale=2.0)


def build_general_nc(nstrip=C * (H // 128)):
    """Per-core program: full 64-plane parity table (see baseline)."""
    import concourse.bass as bass
    import concourse.tile as tile
    from concourse import mybir

    f32 = mybir.dt.float32
    nc = bass.Bass()
    x = nc.dram_tensor("x", [C, H, W], f32, kind="ExternalInput")
    br = nc.dram_tensor("br", [128, 128], f32, kind="ExternalInput")
    bc = nc.dram_tensor("bc", [128, 128], f32, kind="ExternalInput")
    o = nc.dram_tensor("o", [nstrip, 128, 1024], f32, kind="ExternalOutput")

    with tile.TileContext(nc) as tc:
        with (
            tc.tile_pool(name="consts", bufs=1) as consts,
            tc.tile_pool(name="xs", bufs=2) as xpool,
            tc.tile_pool(name="ysb", bufs=2) as ypool,
            tc.tile_pool(name="pk", bufs=2) as pkpool,
            tc.tile_pool(name="yp", bufs=4, space="PSUM") as yppool,
            tc.tile_pool(name="fp", bufs=4, space="PSUM") as fppool,
        ):
            brt = consts.tile([128, 128], f32)
            nc.sync.dma_start(out=brt[:], in_=br[:, :])
            bct = consts.tile([128, 128], f32)
            nc.sync.dma_start(out=bct[:], in_=bc[:, :])

            for s in range(nstrip):
                c, hg = divmod(s, H // 128)
                xs = xpool.tile([128, 1024], f32, tag="xs")
                nc.sync.dma_start(
                    out=xs[:], in_=x[c, hg * 128:(hg + 1) * 128, :])
                ysb = ypool.tile([128, 1024], f32, tag="ysb")
                for wc in range(8):
                    yp = yppool.tile([128, 128], f32, tag="yp")
                    nc.tensor.matmul(
                        out=yp[:],
                        lhsT=xs[:, wc * 128:(wc + 1) * 128],
                        rhs=brt[:],
                        start=True, stop=True)
                    nc.vector.tensor_copy(
                        out=ysb[:, wc * 128:(wc + 1) * 128], in_=yp[:])
                pk = pkpool.tile([128, 1024], f32, tag="pk")
                hk = pkpool.tile([128, 1024], f32, tag="hk")
                for wc in range(8):
                    fp = fppool.tile([128, 128], f32, tag="fp")
                    nc.tensor.matmul(
                        out=fp[:],
                        lhsT=bct[:],
                        rhs=ysb[:, wc * 128:(wc + 1) * 128],
                        start=True, stop=True)
                    nc.scalar.activation(
                        out=pk[:, wc * 128:(wc + 1) * 128], in_=fp[:],
                        func=mybir.ActivationFunctionType.Abs)
                _parity_ops_general(nc, pk, hk)
                nc.sync.dma_start(out=o[s], in_=pk[:])
    return nc


def _run_spmd(nc, in_maps, trace=False):
    from concourse.bass_utils import run_bass_kernel_spmd

    _split_sync_waits(nc)

    res = run_bass_kernel_spmd(
        nc, in_maps, core_ids=list(range(B)), trace=trace)
    _CACHE["last_results"] = res
    return res.results


def _fast_path(stego, trace=False):
    key = "fast_nc5"
    if key not in _CACHE:
        _CACHE[key] = build_fast_nc()
    nc = _CACHE[key]
    w16, w8 = _CACHE.setdefault("consts_fast5", _build_w_fast())
    in_maps = []
    for b in range(B):
        x16, r8 = _prep_core(stego[b])
        in_maps.append({"x16": x16, "r8": r8, "w16": w16, "w8": w8})
    results = _run_spmd(nc, in_maps, trace=trace)
    out = np.zeros((B, NUM_BITS), dtype=np.float32)
    for b in range(B):
        po = (results[b]["po"].view(np.uint16) != 0).astype(
            np.float32)  # [3, 64, 1024]
        seg = np.ascontiguousarray(
            po.reshape(C, 16, 4, 8, 128).transpose(0, 3, 1, 4, 2)
        ).reshape(-1)
        out[b, b * SEG:(b + 1) * SEG] = seg
    return out


def _general_path(stego, b_idx, c_idx, nh_idx, nw_idx, bh_idx, bw_idx,
                  trace=False):
    key = "general_nc"
    if key not in _CACHE:
        _CACHE[key] = build_general_nc()
    nc = _CACHE[key]
    BR8, BC8 = _CACHE.setdefault("consts_general", _build_consts_general())
    in_maps = [
        {"x": np.ascontiguousarray(stego[b]), "br": BR8, "bc": BC8}
        for b in range(B)
    ]
    results = _run_spmd(nc, in_maps, trace=trace)

    b_idx = np.asarray(b_idx).astype(np.int64)
    c_idx = np.asarray(c_idx).astype(np.int64)
    nh_idx = np.asarray(nh_idx).astype(np.int64)
    nw_idx = np.asarray(nw_idx).astype(np.int64)
    bh_idx = np.asarray(bh_idx).astype(np.int64)
    bw_idx = np.asarray(bw_idx).astype(np.int64)
    num_bits = b_idx.shape[0]

    # table[s=(c,hg), l*16+nwl, wc*128 + nhl*8 + i]
    s = c_idx * 8 + nh_idx // 16
    part = bw_idx * 16 + nw_idx % 16
    free = (nw_idx // 16) * 128 + (nh_idx % 16) * 8 + bh_idx
    flat = (s * 128 + part) * 1024 + free

    out = np.zeros((B, num_bits), dtype=np.float32)
    cols = np.arange(num_bits)
    for b in range(B):
        tb = results[b]["o"].reshape(-1)
        mask = b_idx == b
        out[b, cols[mask]] = tb[flat[mask]]
    return out


def kernel(stego, b_idx, c_idx, nh_idx, nw_idx, bh_idx, bw_idx):
    stego = np.ascontiguousarray(np.asarray(stego, dtype=np.float32))
    import os
    trace = os.environ.get("BASS_TRACE", "") not in ("", "0")
    if _is_canonical(b_idx, c_idx, nh_idx, nw_idx, bh_idx, bw_idx):
        return _fast_path(stego, trace=trace)
    return _general_path(
        stego, b_idx, c_idx, nh_idx, nw_idx, bh_idx, bw_idx, trace=trace)


# revision 60
# speedup vs baseline: 1.1355x; 1.0882x over previous
"""Trainium2 Bass kernel for nn_DCTExtractor.

Reference computation:
  - stego [8, 3, 1024, 1024] f32; per 8x8 block 2D DCT-II (D @ X @ D^T).
  - bits[i] = abs(round_half_even(dct[b,c,nh,nw,bh,bw])) % 2 for 1572864
    index tuples.
  - out [8, num_bits]: out[b_idx[i], i] = bits[i]; other rows 0.

Sharding: data-parallel over batch b across the 8 NeuronCores; core b
processes image b and produces output row b.

Fast path (canonical meshgrid indices): each needed coefficient is a
Frobenius inner product <X_block, D_bh (x) D_bw>.  The kernel is
HBM-bandwidth bound, so the host ships each image as 3 bytes/pixel:
an fp16 hi plane and an fp8e4m3 plane of the residual*128, both in a
layout with the two within-block axes on partitions:

    x16/r8[c, (q,j) 128, (gh, k, g', nw) 8192]   nh = (gh*4+g')*16 + q,
                                                 w = nw*8 + k

The fp16 stationary packs hi/lo weight planes side by side (the lo
plane pre-scaled by 512)

    W16[k][(q,j), 0:64]   = fp16_hi(D[bh_p, j] * D[bw_p, k])
    W16[k][(q,j), 64:128] = fp16_lo residual * 512

so per (c, gh): 8 fp16 matmuls accumulate hi products into PSUM rows
0:64 and 512*lo products into rows 64:128, then 8 fp8 matmuls (W*4 @
r*128 = 512*W@r) accumulate the residual correction into the SAME rows
64:128.  96 matmuls of 512 cols total; PE time ~21 us, well under the
~24 us the 9.3 MB input stream needs at the ~400 GB/s the sync-queue
DMA sustains.  The parity chain is 4 ops: Scalar stages the lo half
out of PSUM (*1/512, exact), Vector folds (hi + lo), adds 1.5*2^23
(pinning the exponent so the f32 mantissa LSB IS the parity of the
RNE-rounded coefficient), a bitwise AND with 1 extracts it, and a
convert-copy emits numeric bf16 0/1 (halving the output stream).

All DMA rides the sync queue in need-order (gh-major, so each
half-channel's accumulation closes and drains while the next half
streams; the last 256 KB piece gates only a few short matmuls + the
split-tail chains).  The head is the weights followed by ONE 1 MB
8KB-line piece: small head descriptors underfeed the 16 DMA engines
through their ramp.

General fallback (arbitrary indices): device computes the full 64-plane
parity table per image; host gathers bits and applies the b mask.
"""

import sys

if "/opt/trn_rl_repo" not in sys.path:
    sys.path.insert(0, "/opt/trn_rl_repo")

import numpy as np
import ml_dtypes

BS = 8
B, C, H, W = 8, 3, 1024, 1024
NBH, NBW = H // BS, W // BS
POS = np.array([[1, 2], [2, 1], [2, 2], [3, 1]], dtype=np.int32)
NPOS = 4
SEG = C * NBH * NBW * NPOS  # bits per batch element = 196608
NUM_BITS = B * SEG
MAGIC = float(np.float32(8388608.0))  # 2^23 (general path, abs first)
M15 = float(np.float32(12582912.0))  # 1.5*2^23: RNE for signed values
IP = [0, 1, 1, 2]  # i' = bh-1 per p
LP = [2, 1, 2, 1]  # l  = bw   per p

_CACHE = {}


def _split_sync_waits(nc):
    """The staged walrus build accepts at most ONE sync wait per
    instruction, but Tile's wait-assignment freely attaches several.
    Hoist all but the last wait of each instruction onto same-engine
    NoOps inserted directly before it (engines execute their stream in
    order, so the semantics are identical)."""
    from concourse import mybir

    if getattr(nc, "_sync_waits_split", False):
        return
    nc._sync_waits_split = True
    counter = 0
    for bb in nc.m.functions[0].blocks:
        out = []
        changed = False
        for inst in bb.instructions:
            si = inst.sync_info
            waits = list(si.on_wait) if si is not None else []
            if len(waits) > 1:
                for w in waits[:-1]:
                    nop = mybir.InstNoOp(
                        name=f"I-splitw-{counter}", ins=[], outs=[])
                    counter += 1
                    nop.engine = inst.engine
                    nop.sync_info = mybir.SyncInfo(on_update=[], on_wait=[w])
                    out.append(nop)
                si.on_wait = waits[-1:]
                changed = True
            out.append(inst)
        if changed:
            bb.instructions = out
    return


def _dct_matrix_f32() -> np.ndarray:
    k = np.arange(BS)[:, None].astype(np.float64)
    m = np.arange(BS)[None, :].astype(np.float64)
    D = np.cos(np.pi * (2.0 * m + 1.0) * k / (2.0 * BS)) * np.sqrt(2.0 / BS)
    D[0, :] = np.sqrt(1.0 / BS)
    return D.astype(np.float32)


def _canonical_indices():
    b, c, nh, nw, p = np.meshgrid(
        np.arange(B), np.arange(C), np.arange(NBH), np.arange(NBW),
        np.arange(NPOS), indexing="ij")
    return {
        "b_idx": b.reshape(-1).astype(np.int32),
        "c_idx": c.reshape(-1).astype(np.int32),
        "nh_idx": nh.reshape(-1).astype(np.int32),
        "nw_idx": nw.reshape(-1).astype(np.int32),
        "bh_idx": POS[p.reshape(-1), 0].astype(np.int32),
        "bw_idx": POS[p.reshape(-1), 1].astype(np.int32),
    }


def _is_canonical(b_idx, c_idx, nh_idx, nw_idx, bh_idx, bw_idx) -> bool:
    if b_idx.shape[0] != NUM_BITS:
        return False
    canon = _CACHE.setdefault("canon", _canonical_indices())
    got = {"b_idx": b_idx, "c_idx": c_idx, "nh_idx": nh_idx,
           "nw_idx": nw_idx, "bh_idx": bh_idx, "bw_idx": bw_idx}
    return all(np.array_equal(np.asarray(got[k]), canon[k]) for k in canon)


def _build_w_fast():
    """w16 [128, 1024] fp16: [Wa16|Wb16] hi/lo planes per k;
    w8 [128, 512] fp8e4m3: W*4 per k (the *4 keeps every entry out of
    the e4m3 subnormal range; the matching 1/512 lives in the chain)."""
    D = _dct_matrix_f32()
    Wt = np.zeros((128, 8, 64), dtype=np.float32)
    for k in range(8):
        for q in range(16):
            for j in range(8):
                for p in range(4):
                    Wt[q * 8 + j, k, q * 4 + p] = (
                        D[1 + IP[p], j] * D[LP[p], k])
    Wa = Wt.astype(np.float16)
    # lo plane pre-scaled by 512 so the fp8 residual matmuls (4W @ 128r
    # = 512*W@r) can accumulate into the SAME PSUM rows; the chain
    # divides the whole lo half by 512 (exact).
    Wb = ((Wt - Wa.astype(np.float32)) * 512.0).astype(np.float16)
    w16 = np.ascontiguousarray(
        np.concatenate([Wa, Wb], axis=2)).reshape(128, 1024)
    w8 = np.ascontiguousarray(
        (Wt * 4.0).astype(ml_dtypes.float8_e4m3)).reshape(128, 512)
    return w16, w8


def build_fast_nc():
    """Per-core program: x16 [3, 128, 8192] fp16 + r8 [3, 128, 8192]
    fp8 (residual*128) -> po [3, 64, 1024] bf16 parity bits,
    po[c][q*4+p][g*128+nw]."""
    import concourse.bass as bass
    import concourse.tile as tile
    from concourse import mybir

    f32 = mybir.dt.float32
    f16 = mybir.dt.float16
    f8 = mybir.dt.float8e4
    bf = mybir.dt.bfloat16
    nc = bass.Bass()
    x16 = nc.dram_tensor("x16", [C, 128, 8192], f16, kind="ExternalInput")
    r8 = nc.dram_tensor("r8", [C, 128, 8192], f8, kind="ExternalInput")
    w16 = nc.dram_tensor("w16", [128, 1024], f16, kind="ExternalInput")
    w8 = nc.dram_tensor("w8", [128, 512], f8, kind="ExternalInput")
    po = nc.dram_tensor("po", [C, 64, 1024], bf, kind="ExternalOutput")

    add = mybir.AluOpType.add
    mult = mybir.AluOpType.mult

    band = mybir.AluOpType.bitwise_and
    i32 = mybir.dt.int32

    def chain(acc, tmp, par_dst, one_col):
        """parity of RNE(acc[0:64] + acc[64:128]/512) -> par_dst
        (bf16 0/1; host maps nonzero->1 for robustness).  acc
        rows 0:64 hi-plane, rows 64:128 hold 512*(lo-plane + W@r).
        s + M15 = M15 + rne(s) exactly (1.5*2^23 pins the exponent for
        |s| < 2^22), so the f32 mantissa LSB IS the parity; an int32
        bitwise-and with 1 extracts it.  Only one PSUM operand per
        instruction is allowed, hence the staging Copy (1/512 exact)."""
        n = par_dst.free_size()
        s = tmp.tile([64, n], f32, tag="s", bufs=3)
        u = tmp.tile([64, n], f32, tag="u", bufs=3)
        nc.scalar.activation(
            out=u[:], in_=acc[64:128, :],
            func=mybir.ActivationFunctionType.Copy, scale=1.0 / 512.0)
        nc.vector.tensor_tensor(
            out=s[:], in0=acc[0:64, :], in1=u[:], op=add)
        nc.vector.tensor_scalar(out=u[:], in0=s[:], scalar1=M15,
                                scalar2=None, op0=add)
        nc.vector.tensor_scalar(out=s[:].bitcast(i32),
                                in0=u[:].bitcast(i32), scalar1=one_col,
                                scalar2=None, op0=band)
        # int32 0/1 -> numeric bf16 0.0/1.0: halves the po stream (the
        # host still maps nonzero->1, so truncating-bitcast semantics
        # of the copy would be tolerated too)
        nc.vector.tensor_copy(out=par_dst, in_=s[:].bitcast(i32))

    with tile.TileContext(nc) as tc:
        with (
            tc.tile_pool(name="sb", bufs=1) as sb,
            tc.tile_pool(name="ps", bufs=8, space="PSUM") as ps,
        ):
            wt16 = sb.tile([128, 1024], f16, tag="w16")
            wt8 = sb.tile([128, 512], f8, tag="w8")
            ones = sb.tile([128, 1], i32, tag="ones")
            nc.vector.memset(ones[:], 1)
            one_col = ones[0:64, :]

            # Input stream on the sync queue in need-order, gh-major so
            # each half-channel's accumulation closes (and its chain +
            # output drain) while the next half streams.
            xtiles = []
            for c in range(C):
                xt = sb.tile([128, 8192], f16, tag="x16", bufs=3,
                             name=f"xt{c}")
                rt = sb.tile([128, 8192], f8, tag="r8", bufs=3,
                             name=f"rt{c}")
                xtiles.append((xt, rt))
                for gh in range(2):
                    for j in range(2):  # 2048-col fp16 pieces (512 KB)
                        sl = slice((gh * 2 + j) * 2048,
                                   (gh * 2 + j + 1) * 2048)
                        if c == 0 and gh == 0:
                            if j == 0:
                                # weights first (their small 2KB-line
                                # descriptors ride the issue-serialized
                                # first us), then ONE large 8KB-line
                                # piece to feed the 16 DMA engines
                                # through their ramp (a small-descriptor
                                # head measured ~240 GB/s)
                                nc.sync.dma_start(out=wt16[:],
                                                  in_=w16[:, :])
                                nc.sync.dma_start(out=wt8[:],
                                                  in_=w8[:, :])
                                nc.sync.dma_start(out=xt[:, 0:4096],
                                                  in_=x16[c][:, 0:4096])
                            continue
                        nc.sync.dma_start(out=xt[:, sl], in_=x16[c][:, sl])
                    for j in range(2):  # 2048-col fp8 pieces (256 KB):
                        # finer pieces are ISSUE-bound (~530 ns/descriptor
                        # on the sync engine vs ~640 ns transfer)
                        sl = slice((gh * 2 + j) * 2048,
                                   (gh * 2 + j + 1) * 2048)
                        nc.sync.dma_start(out=rt[:, sl], in_=r8[c][:, sl])

            for c in range(C):
                xt, rt = xtiles[c]
                par_c = sb.tile([64, 1024], bf, tag="par", bufs=3)
                for gh in range(2):
                    if c == 2 and gh == 1:
                        break  # handled below with a split tail
                    acc = ps.tile([128, 512], f32, tag="ps",
                                  name=f"acc{c}{gh}", bufs=6)
                    for k in range(8):
                        base = gh * 4096 + k * 512
                        nc.tensor.matmul(
                            out=acc[:, :],
                            lhsT=wt16[:, k * 128:(k + 1) * 128],
                            rhs=xt[:, base:base + 512],
                            start=(k == 0), stop=False)
                    for k in range(8):
                        base = gh * 4096 + k * 512
                        nc.tensor.matmul(
                            out=acc[64:128, :],
                            lhsT=wt8[:, k * 64:(k + 1) * 64],
                            rhs=rt[:, base:base + 512],
                            start=False, stop=(k == 7))
                    chain(acc[:], sb, par_c[:, gh * 512:(gh + 1) * 512],
                          one_col)
                    nc.sync.dma_start(
                        out=po[c][:, gh * 512:(gh + 1) * 512],
                        in_=par_c[:, gh * 512:(gh + 1) * 512])

            # The very last half-channel (c2, gh1) runs as TWO column-
            # half accumulation groups so that after the final 256 KB
            # r8 piece lands, half A closes in 4 short matmuls and its
            # chain overlaps half B's matmuls — ~1 us off the tail.
            xt, rt = xtiles[2]
            accs = [ps.tile([128, 256], f32, tag="ps2",
                            name=f"accL{h}", bufs=2) for h in range(2)]
            for k in range(8):
                base = 4096 + k * 512
                for h in range(2):
                    nc.tensor.matmul(
                        out=accs[h][:, :],
                        lhsT=wt16[:, k * 128:(k + 1) * 128],
                        rhs=xt[:, base + h * 256:base + (h + 1) * 256],
                        start=(k == 0), stop=False)
            for k in range(4):
                base = 4096 + k * 512
                for h in range(2):
                    nc.tensor.matmul(
                        out=accs[h][64:128, :],
                        lhsT=wt8[:, k * 64:(k + 1) * 64],
                        rhs=rt[:, base + h * 256:base + (h + 1) * 256],
                        start=False, stop=False)
            for h in range(2):  # gated by the final piece: h-major
                for k in range(4, 8):
                    base = 4096 + k * 512
                    nc.tensor.matmul(
                        out=accs[h][64:128, :],
                        lhsT=wt8[:, k * 64:(k + 1) * 64],
                        rhs=rt[:, base + h * 256:base + (h + 1) * 256],
                        start=False, stop=(k == 7))
            for h in range(2):
                sl = slice(512 + h * 256, 512 + (h + 1) * 256)
                chain(accs[h][:], sb, par_c[:, sl], one_col)
                nc.sync.dma_start(out=po[2][:, sl], in_=par_c[:, sl])
    return nc


def _prep_core(x):
    """x [3,1024,1024] f32 -> (x16 fp16, r8 fp8e4m3 of residual*128) in
    the (q,j)-partition, (gh, k, g', nw)-free layout (g = gh*4+g')."""
    x16 = x.astype(np.float16)
    r = x - x16.astype(np.float32)
    r8 = (r * 128.0).astype(ml_dtypes.float8_e4m3)

    def rl(a):
        # (c, gh, g', q, j, nw, k) -> (c, q, j, gh, k, g', nw)
        return np.ascontiguousarray(
            a.reshape(C, 2, 4, 16, 8, 128, 8).transpose(0, 3, 4, 1, 6, 2, 5)
        ).reshape(C, 128, 8192)

    return rl(x16), rl(r8)


def _build_consts_general():
    D = _dct_matrix_f32()
    BR8 = np.zeros((128, 128), dtype=np.float32)
    for nhl in range(16):
        BR8[nhl * 8:(nhl + 1) * 8, nhl * 8:(nhl + 1) * 8] = D.T  # [j, i]
    BC8 = np.zeros((128, 128), dtype=np.float32)
    for l in range(8):
        for nwl in range(16):
            BC8[nwl * 8:(nwl + 1) * 8, l * 16 + nwl] = D[l, :]
    return BR8, BC8


def _parity_ops_general(nc, pk, hk):
    """pk holds |coeff| >= 0; parity via 2^23 magic (each step its own
    instruction so every intermediate is rounded f32)."""
    from concourse import mybir

    ts = nc.vector.tensor_scalar
    add, sub, mult = (mybir.AluOpType.add, mybir.AluOpType.subtract,
                      mybir.AluOpType.mult)
    ts(out=pk[:], in0=pk[:], scalar1=MAGIC, scalar2=None, op0=add)
    ts(out=pk[:], in0=pk[:], scalar1=MAGIC, scalar2=None, op0=sub)
    ts(out=hk[:], in0=pk[:], scalar1=0.5, scalar2=None, op0=mult)
    ts(out=pk[:], in0=hk[:], scalar1=MAGIC, scalar2=None, op0=add)
    ts(out=pk[:], in0=pk[:], scalar1=MAGIC, scalar2=None, op0=sub)
    nc.vector.tensor_tensor(out=pk[:], in0=hk[:], in1=pk[:], op=sub)
    nc.scalar.activation(
        out=pk[:], in_=pk[:], func=mybir.ActivationFunctionType.Abs,
        scale=2.0)


def build_general_nc(nstrip=C * (H // 128)):
    """Per-core program: full 64-plane parity table (see baseline)."""
    import concourse.bass as bass
    import concourse.tile as tile
    from concourse import mybir

    f32 = mybir.dt.float32
    nc = bass.Bass()
    x = nc.dram_tensor("x", [C, H, W], f32, kind="ExternalInput")
    br = nc.dram_tensor("br", [128, 128], f32, kind="ExternalInput")
    bc = nc.dram_tensor("bc", [128, 128], f32, kind="ExternalInput")
    o = nc.dram_tensor("o", [nstrip, 128, 1024], f32, kind="ExternalOutput")

    with tile.TileContext(nc) as tc:
        with (
            tc.tile_pool(name="consts", bufs=1) as consts,
            tc.tile_pool(name="xs", bufs=2) as xpool,
            tc.tile_pool(name="ysb", bufs=2) as ypool,
            tc.tile_pool(name="pk", bufs=2) as pkpool,
            tc.tile_pool(name="yp", bufs=4, space="PSUM") as yppool,
            tc.tile_pool(name="fp", bufs=4, space="PSUM") as fppool,
        ):
            brt = consts.tile([128, 128], f32)
            nc.sync.dma_start(out=brt[:], in_=br[:, :])
            bct = consts.tile([128, 128], f32)
            nc.sync.dma_start(out=bct[:], in_=bc[:, :])

            for s in range(nstrip):
                c, hg = divmod(s, H // 128)
                xs = xpool.tile([128, 1024], f32, tag="xs")
                nc.sync.dma_start(
                    out=xs[:], in_=x[c, hg * 128:(hg + 1) * 128, :])
                ysb = ypool.tile([128, 1024], f32, tag="ysb")
                for wc in range(8):
                    yp = yppool.tile([128, 128], f32, tag="yp")
                    nc.tensor.matmul(
                        out=yp[:],
                        lhsT=xs[:, wc * 128:(wc + 1) * 128],
                        rhs=brt[:],
                        start=True, stop=True)
                    nc.vector.tensor_copy(
                        out=ysb[:, wc * 128:(wc + 1) * 128], in_=yp[:])
                pk = pkpool.tile([128, 1024], f32, tag="pk")
                hk = pkpool.tile([128, 1024], f32, tag="hk")
                for wc in range(8):
                    fp = fppool.tile([128, 128], f32, tag="fp")
                    nc.tensor.matmul(
                        out=fp[:],
                        lhsT=bct[:],
                        rhs=ysb[:, wc * 128:(wc + 1) * 128],
                        start=True, stop=True)
                    nc.scalar.activation(
                        out=pk[:, wc * 128:(wc + 1) * 128], in_=fp[:],
                        func=mybir.ActivationFunctionType.Abs)
                _parity_ops_general(nc, pk, hk)
                nc.sync.dma_start(out=o[s], in_=pk[:])
    return nc


def _run_spmd(nc, in_maps, trace=False):
    from concourse.bass_utils import run_bass_kernel_spmd

    _split_sync_waits(nc)

    res = run_bass_kernel_spmd(
        nc, in_maps, core_ids=list(range(B)), trace=trace)
    _CACHE["last_results"] = res
    return res.results


def _fast_path(stego, trace=False):
    key = "fast_nc5"
    if key not in _CACHE:
        _CACHE[key] = build_fast_nc()
    nc = _CACHE[key]
    w16, w8 = _CACHE.setdefault("consts_fast5", _build_w_fast())
    in_maps = []
    for b in range(B):
        x16, r8 = _prep_core(stego[b])
        in_maps.append({"x16": x16, "r8": r8, "w16": w16, "w8": w8})
    results = _run_spmd(nc, in_maps, trace=trace)
    out = np.zeros((B, NUM_BITS), dtype=np.float32)
    for b in range(B):
        po = (results[b]["po"].view(np.uint16) != 0).astype(
            np.float32)  # [3, 64, 1024]
        seg = np.ascontiguousarray(
            po.reshape(C, 16, 4, 8, 128).transpose(0, 3, 1, 4, 2)
        ).reshape(-1)
        out[b, b * SEG:(b + 1) * SEG] = seg
    return out


def _general_path(stego, b_idx, c_idx, nh_idx, nw_idx, bh_idx, bw_idx,
                  trace=False):
    key = "general_nc"
    if key not in _CACHE:
        _CACHE[key] = build_general_nc()
    nc = _CACHE[key]
    BR8, BC8 = _CACHE.setdefault("consts_general", _build_consts_general())
    in_maps = [
        {"x": np.ascontiguousarray(stego[b]), "br": BR8, "bc": BC8}
        for b in range(B)
    ]
    results = _run_spmd(nc, in_maps, trace=trace)

    b_idx = np.asarray(b_idx).astype(np.int64)
    c_idx = np.asarray(c_idx).astype(np.int64)
    nh_idx = np.asarray(nh_idx).astype(np.int64)
    nw_idx = np.asarray(nw_idx).astype(np.int64)
    bh_idx = np.asarray(bh_idx).astype(np.int64)
    bw_idx = np.asarray(bw_idx).astype(np.int64)
    num_bits = b_idx.shape[0]

    # table[s=(c,hg), l*16+nwl, wc*128 + nhl*8 + i]
    s = c_idx * 8 + nh_idx // 16
    part = bw_idx * 16 + nw_idx % 16
    free = (nw_idx // 16) * 128 + (nh_idx % 16) * 8 + bh_idx
    flat = (s * 128 + part) * 1024 + free

    out = np.zeros((B, num_bits), dtype=np.float32)
    cols = np.arange(num_bits)
    for b in range(B):
        tb = results[b]["o"].reshape(-1)
        mask = b_idx == b
        out[b, cols[mask]] = tb[flat[mask]]
    return out


def kernel(stego, b_idx, c_idx, nh_idx, nw_idx, bh_idx, bw_idx):
    stego = np.ascontiguousarray(np.asarray(stego, dtype=np.float32))
    import os
    trace = os.environ.get("BASS_TRACE", "") not in ("", "0")
    if _is_canonical(b_idx, c_idx, nh_idx, nw_idx, bh_idx, bw_idx):
        return _fast_path(stego, trace=trace)
    return _general_path(
        stego, b_idx, c_idx, nh_idx, nw_idx, bh_idx, bw_idx, trace=trace)
